# revision 2
# baseline (speedup 1.0000x reference)
"""ECE loss kernel for Trainium2, data-parallel over 8 NeuronCores.

Strategy (v2 — fp16 streaming)
------------------------------
ECE = sum_b |sum_{i in bin b} (conf_i - acc_i)| / N, so the only binned
statistic needed per bin is d_b = sum(conf - acc).

The softmaxes are converted to fp16 on the host (rel err of the final
ECE ~1e-5, tolerance 2e-2), halving HBM traffic; the per-NC HBM roofline
(~358 GB/s) then gives ~190 us for the 67 MB/core stream.

Per core (N/8 = 524288 samples as 32 tiles of [128 part x 128 samp x 64 cl]):

1. Host lays each tile out class-major as two 32-class halves A and B.
   DMA A plain, then DMA B onto the same SBUF buffer with accum_op=max:
   the SDMA CCE ALU computes the elementwise max in-flight, absorbing the
   first max-tree level into the (otherwise idle) DMA engines.
   A per-tile flag (DMA_MAX) can fall back to plain B + a DVE max level.
2. DVE: 5 stock tensor_tensor MAX levels (fp16 -> 2 elem/cycle/lane)
   reduce 32 -> 1: conf16 per sample.  argmax is never materialized:
   acc_i = (conf16_i == xlab16_i) where xlab16[i] = sm16[i, label_i] is
   gathered on the host (equality of fp16 values; ~26/4.2M flips vs the
   fp32 reference argmax).  z = conf - acc via one more TT pass.
3. 15 custom BIN_RANGE_SUM passes per group: accum_out[p, b] +=
   sum(z where lo < conf <= hi), 1 elem/cycle (fused spec).
4. Host: sum per-core/per-partition stats in float64, abs, sum, /N.
"""

import sys

for _p in ("/opt/trn_rl_repo",):
    if _p not in sys.path:
        sys.path.insert(0, _p)

import numpy as np

import concourse.bass as bass
import concourse.mybir as mybir
import concourse.dve_spec as ds
import concourse.dve_ops as dops
from concourse.dve_spec import Spec, Src0, Src1, Zero, AluOp, lower, select
from concourse.dve_uop import DveOpSpec
from concourse.dve_ops import DveOp, OPS
from concourse.bass_utils import run_bass_kernel_spmd

# ----------------------------------------------------------------------------
# problem constants (hardcoded per the harness contract)
# ----------------------------------------------------------------------------
N_TOTAL = 4194304
C = 64
HALF = C // 2
N_BINS = 15
CORES = 8
NC_SAMP = N_TOTAL // CORES        # 524288 samples per core
P = 128                           # SBUF partitions
S_TILE = 128                      # samples per partition per tile
T = NC_SAMP // (P * S_TILE)       # 32 tiles per core
SPT = T * S_TILE                  # samples per partition per core (4096)
FA = HALF * S_TILE                # elems per partition per half-tile (4096)

# binning groups (tile counts); last group kept tiny to shrink the drain tail
GROUP_TILES = [15, 16, 1]
NG = len(GROUP_TILES)

SM_BUFS = 8                       # softmax tile ring depth
# which tiles get their B-half max-reduced by the DMA CCE (vs on DVE)
DMA_MAX = [True] * T

BOUNDS = np.linspace(0.0, 1.0, N_BINS + 1).astype(np.float32)

# ----------------------------------------------------------------------------
# custom DVE op: BIN_RANGE_SUM: out = (C0 < Src0 <= C1) ? Src1 : 0;
# accum_out = sum(out)
# ----------------------------------------------------------------------------


def _make_op(name, spec_body, reference, subdim, accum=None):
    spec_kw = dict(body=spec_body, reference=reference)
    if accum is not None:
        spec_kw["accum"] = accum
    spec = Spec(**spec_kw)
    shas = {}
    for ver in ("v3", "v4"):
        uops = lower(spec, ver=ver)
        shas[ver] = DveOpSpec(
            name=name, opcode=0, uops=uops, rd1_en=ds._has_src1(spec)
        ).sha(ver)
    op = DveOp(name, spec, subdim=subdim, uops_sha=shas)
    if name not in dops._SUB_OPCODE_FOR_NAME:
        OPS.append(op)
        dops.CUSTOM_DVE_SPECS[name] = spec
        dops._SUB_OPCODE_FOR_NAME[name] = dops._CUSTOM_DVE_ROW_BASE + len(OPS) - 1
        assert dops._SUB_OPCODE_FOR_NAME[name] < 0x20
    else:
        op = next(o for o in OPS if o.name == name)
    return op


_inbin = ds.Bin(AluOp.LOGICAL_AND, Src0 > ds.C0, Src0 <= ds.C1)
_body2 = select(_inbin, Src1, Zero)


def _bin_range_sum_ref(in0, in1, s0, s1, imm2):
    x = np.asarray(in0, np.float32)
    z = np.asarray(in1, np.float32)
    out = np.where((x > s0) & (x <= s1), z, 0.0).astype(np.float32)
    acc = out.reshape(out.shape[0], -1).sum(axis=-1, keepdims=True).astype(np.float32)
    return out, acc


BIN_RANGE_SUM = _make_op(
    "BIN_RANGE_SUM_ANT", _body2, _bin_range_sum_ref, subdim=False, accum=AluOp.ADD
)

# ----------------------------------------------------------------------------
# bass program (one NEFF, run SPMD on 8 cores)
# ----------------------------------------------------------------------------
f16 = mybir.dt.float16
f32 = mybir.dt.float32

_NC_CACHE = {}


def _build_nc(repeats: int = 1, variant: str = "full"):
    """Raw Bass (no Tile).  repeats > 1 re-runs the identical workload
    back-to-back (for timing); every repeat recomputes the same dstat
    values, so results are unchanged.
    variant: "full" (normal), "dma" (loads only), "dve" (compute only) —
    the last two are roofline micro-benchmarks.
    """
    key = (repeats, variant)
    if key in _NC_CACHE:
        return _NC_CACHE[key]
    nc = bass.Bass()
    sm = nc.dram_tensor("sm", [T, P, C * S_TILE], f16, kind="ExternalInput")
    xlab = nc.dram_tensor("xlab", [P, SPT], f16, kind="ExternalInput")
    dstat = nc.dram_tensor("dstat", [P, NG * 16], f32, kind="ExternalOutput")

    sm_v = sm.ap()

    smt = [
        nc.alloc_sbuf_tensor(f"smt{i}", [P, C * S_TILE], f16).ap()
        for i in range(SM_BUFS)
    ]
    m1 = nc.alloc_sbuf_tensor("m1", [P, FA], f16).ap()
    m2 = nc.alloc_sbuf_tensor("m2", [P, FA // 2], f16).ap()
    m3 = nc.alloc_sbuf_tensor("m3", [P, FA // 4], f16).ap()
    m4 = nc.alloc_sbuf_tensor("m4", [P, FA // 8], f16).ap()
    conf_sb = nc.alloc_sbuf_tensor("conf_sb", [P, SPT], f16).ap()
    xlab_sb = nc.alloc_sbuf_tensor("xlab_sb", [P, SPT], f16).ap()
    sg_max = max(GROUP_TILES) * S_TILE
    acc_sb = nc.alloc_sbuf_tensor("acc_sb", [P, sg_max], f16).ap()
    z_sb = nc.alloc_sbuf_tensor("z_sb", [P, sg_max], f16).ap()
    dstat_sb = nc.alloc_sbuf_tensor("dstat_sb", [P, NG * 16], f32).ap()
    scrap = nc.alloc_sbuf_tensor("scrap", [P, 1], f16).ap()

    dsem = nc.alloc_semaphore()   # B-half DMA completions (+16 each)
    xsem = nc.alloc_semaphore()   # xlab DMA completion
    vsem = nc.alloc_semaphore()   # DVE tile consumption (+1 per sm tile)
    done = nc.alloc_semaphore()   # DVE fully done (+1 per repeat)

    do_dma = variant in ("full", "dma")
    do_dve = variant in ("full", "dve")
    gated = variant == "full"

    mx = mybir.AluOpType.max

    # ---- SP (sync) engine: aux DMAs ----
    nc.sync.dma_start(xlab_sb[:], xlab.ap()[:]).then_inc(xsem, 16)
    if gated or variant == "dve":
        nc.sync.wait_ge(done, repeats)
    elif variant == "dma":
        nc.sync.wait_ge(dsem, 16 * T * repeats)
    nc.sync.dma_start(dstat.ap()[:], dstat_sb[:]).then_inc(xsem, 16)
    nc.sync.wait_ge(xsem, 32)

    # ---- gpsimd (Pool) engine: softmax stream (SWDGE; accum-capable) ----
    if do_dma:
        for r in range(repeats):
            for t in range(T):
                i = r * T + t
                if gated and i >= SM_BUFS:
                    nc.gpsimd.wait_ge(vsem, i - SM_BUFS + 1)
                buf = smt[i % SM_BUFS]
                if DMA_MAX[t]:
                    nc.gpsimd.dma_start(buf[:, 0:FA], sm_v[t][:, 0:FA])
                    nc.gpsimd.dma_start(
                        buf[:, 0:FA], sm_v[t][:, FA : 2 * FA], accum_op=mx
                    ).then_inc(dsem, 16)
                else:
                    nc.gpsimd.dma_start(buf[:, 0:FA], sm_v[t][:, 0:FA])
                    nc.gpsimd.dma_start(
                        buf[:, FA : 2 * FA], sm_v[t][:, FA : 2 * FA]
                    ).then_inc(dsem, 16)

    # ---- DVE program ----
    def tt(out, a, b, op):
        return nc.vector.tensor_tensor(out=out, in0=a, in1=b, op=op)

    def bins_for_group(g, g0_tile, ntiles):
        """eq/z + 15 bin passes over conf_sb[:, group slice]."""
        sg = ntiles * S_TILE
        lo_c = g0_tile * S_TILE
        csl = conf_sb[:, lo_c : lo_c + sg]
        tt(acc_sb[:, 0:sg], csl, xlab_sb[:, lo_c : lo_c + sg],
           mybir.AluOpType.is_equal)
        tt(z_sb[:, 0:sg], csl, acc_sb[:, 0:sg], mybir.AluOpType.subtract)
        inst = None
        for b in range(N_BINS):
            lo = float(BOUNDS[b])
            hi = 1.001 if b == N_BINS - 1 else float(BOUNDS[b + 1])
            inst = nc.vector._custom_dve(
                BIN_RANGE_SUM,
                out=scrap[:].broadcast_to([P, sg]),
                accum_out=dstat_sb[:, g * 16 + b : g * 16 + b + 1],
                in0=csl,
                in1=z_sb[:, 0:sg],
                s0=lo,
                s1=hi,
            )
        return inst

    if do_dve:
        # group boundaries
        g_start = np.cumsum([0] + GROUP_TILES).tolist()
        for r in range(repeats):
            for t in range(T):
                i = r * T + t
                buf = smt[i % SM_BUFS]
                if gated:
                    nc.vector.wait_ge(dsem, 16 * (i + 1))
                if DMA_MAX[t]:
                    lvl = tt(m2[:], buf[:, 0 : FA // 2],
                             buf[:, FA // 2 : FA], mx)
                else:
                    lvl = tt(m1[:], buf[:, 0:FA], buf[:, FA : 2 * FA], mx)
                    lvl.then_inc(vsem, 1)
                    lvl = tt(m2[:], m1[:, 0 : FA // 2], m1[:, FA // 2 : FA], mx)
                if DMA_MAX[t]:
                    lvl.then_inc(vsem, 1)
                tt(m3[:], m2[:, 0 : FA // 4], m2[:, FA // 4 : FA // 2], mx)
                tt(m4[:], m3[:, 0 : FA // 8], m3[:, FA // 8 : FA // 4], mx)
                cs = conf_sb[:, t * S_TILE : (t + 1) * S_TILE]
                tt(cs, m4[:, 0 : S_TILE], m4[:, S_TILE : 2 * S_TILE], mx)
                # group end?
                for g in range(NG):
                    if t == g_start[g + 1] - 1:
                        if r == 0 and g == 0 and gated:
                            nc.vector.wait_ge(xsem, 16)
                        inst = bins_for_group(g, g_start[g], GROUP_TILES[g])
                        if g == NG - 1:
                            inst.then_inc(done, 1)

    # Raw Bass skips this pass; without it InstCustomDveAnt/.instr stays
    # empty and walrus fails with "ISA wrong length".
    mybir.codegen_inst_isa_subclasses(nc)
    _NC_CACHE[key] = nc
    return nc


# ----------------------------------------------------------------------------
# host-side input prep (layout only: fp16 convert, permute, label gather)
# ----------------------------------------------------------------------------
def prepare_inputs(softmaxes: np.ndarray, labels: np.ndarray):
    sm16 = np.asarray(softmaxes, dtype=np.float32).astype(np.float16)
    assert sm16.shape == (N_TOTAL, C), sm16.shape
    lab = np.asarray(labels).astype(np.int64).reshape(N_TOTAL)

    xl = sm16[np.arange(N_TOTAL), lab]                       # [N] f16
    xv = np.ascontiguousarray(
        xl.reshape(CORES, T, P, S_TILE).transpose(0, 2, 1, 3)
    ).reshape(CORES, P, SPT)

    v = sm16.reshape(CORES, T, P, S_TILE, C)
    smc = np.ascontiguousarray(v.transpose(0, 1, 2, 4, 3)).reshape(
        CORES, T, P, C * S_TILE
    )

    in_maps = []
    for k in range(CORES):
        in_maps.append({"sm": smc[k], "xlab": xv[k]})
    return in_maps


def finish(results) -> np.ndarray:
    d = np.zeros(N_BINS, np.float64)
    for k in range(CORES):
        st = results[k]["dstat"].astype(np.float64)
        st = st.reshape(P, NG, 16)[:, :, :N_BINS]
        d += st.sum(axis=(0, 1))
    ece = np.float32(np.abs(d).sum() / N_TOTAL)
    return np.array([ece], dtype=np.float32)


# ----------------------------------------------------------------------------
# public entry point
# ----------------------------------------------------------------------------
def kernel(softmaxes: np.ndarray, labels: np.ndarray, _want_trace=False, _repeats=1):
    nc = _build_nc(_repeats)
    in_maps = prepare_inputs(softmaxes, labels)
    res = run_bass_kernel_spmd(nc, in_maps, core_ids=list(range(CORES)))
    out = finish(res.results)
    if _want_trace:
        return out, res
    return out


# revision 7
# speedup vs baseline: 1.7964x; 1.7964x over previous
"""ECE loss kernel for Trainium2, data-parallel over 8 NeuronCores.

Strategy (v2 — fp16 streaming)
------------------------------
ECE = sum_b |sum_{i in bin b} (conf_i - acc_i)| / N, so the only binned
statistic needed per bin is d_b = sum(conf - acc).

The softmaxes are converted to fp16 on the host (rel err of the final
ECE ~1e-5, tolerance 2e-2), halving HBM traffic vs fp32.  Host also lays
each per-core stream out class-major per tile and gathers
xlab16[i] = sm16[i, label_i] (pure layout/indexing; all reduction math
stays on device).

Per core (N/8 = 524288 samples as 16 tiles of [128 part x 256 samp x 64 cl]):

1. DMA the tile (two 32-class halves, contiguous per partition).
2. DVE: 6 stock tensor_tensor MAX levels (fp16 -> 2 elem/cycle/lane)
   reduce 64 -> 1: conf16 per sample.  argmax is never materialized:
   acc_i = (conf16_i == xlab16_i) (equality of fp16 values; ~26/4.2M
   flips vs the fp32 reference argmax).
3. 15 fused BIN_RANGE_ZSUM passes per tile-group: per element,
   z = conf - (conf == xlab), accum_out[p] += z where lo < conf <= hi.
   One DVE pass per bin at 1 elem/cycle; eq/sub are folded in (free).
4. Host: sum the per-core/per-partition stats in float64, abs, sum, /N.
"""

import sys

for _p in ("/opt/trn_rl_repo",):
    if _p not in sys.path:
        sys.path.insert(0, _p)

import numpy as np

import concourse.bass as bass
import concourse.mybir as mybir
import concourse.dve_spec as ds
from concourse.dve_spec import Spec, Src0, Src1, Zero, AluOp, lower, select, eq
from concourse.dve_uop import DveOpSpec
from concourse.dve_ops import DveOp, OPS
import concourse.dve_ops as dops
from concourse.bass_utils import run_bass_kernel_spmd

# ----------------------------------------------------------------------------
# problem constants (hardcoded per the harness contract)
# ----------------------------------------------------------------------------
N_TOTAL = 4194304
C = 64
HALF = C // 2
N_BINS = 15
CORES = 8
NC_SAMP = N_TOTAL // CORES        # 524288 samples per core
P = 128                           # SBUF partitions
S_TILE = 256                      # samples per partition per tile
T = NC_SAMP // (P * S_TILE)       # 16 tiles per core
SPT = T * S_TILE                  # samples per partition per core (4096)
FT = C * S_TILE                   # elems per partition per tile (16384)
FA = HALF * S_TILE                # elems per partition per half-tile (8192)

GROUP_TILES = [8, 7, 1]           # binning groups; tiny last group = short tail
NG = len(GROUP_TILES)
SM_BUFS = 4                       # softmax tile ring depth (4 x 32KB/partition)

BOUNDS = np.linspace(0.0, 1.0, N_BINS + 1).astype(np.float32)

# ----------------------------------------------------------------------------
# custom DVE op: BIN_RANGE_ZSUM
#   out   = (C0 < Src0 <= C1) ? (Src0 - (Src0 == Src1)) : 0
#   accum_out = sum(out)     [Src0 = conf, Src1 = softmax at the true label]
# ----------------------------------------------------------------------------


def _make_op(name, spec_body, reference, subdim, accum=None):
    spec_kw = dict(body=spec_body, reference=reference)
    if accum is not None:
        spec_kw["accum"] = accum
    spec = Spec(**spec_kw)
    shas = {}
    for ver in ("v3", "v4"):
        uops = lower(spec, ver=ver)
        shas[ver] = DveOpSpec(
            name=name, opcode=0, uops=uops, rd1_en=ds._has_src1(spec)
        ).sha(ver)
    op = DveOp(name, spec, subdim=subdim, uops_sha=shas)
    if name not in dops._SUB_OPCODE_FOR_NAME:
        OPS.append(op)
        dops.CUSTOM_DVE_SPECS[name] = spec
        dops._SUB_OPCODE_FOR_NAME[name] = dops._CUSTOM_DVE_ROW_BASE + len(OPS) - 1
        assert dops._SUB_OPCODE_FOR_NAME[name] < 0x20
    else:
        op = next(o for o in OPS if o.name == name)
    return op


_inbin = ds.Bin(AluOp.LOGICAL_AND, Src0 > ds.C0, Src0 <= ds.C1)
_body = select(_inbin, Src0 - eq(Src0, Src1), Zero)


def _bin_range_zsum_ref(in0, in1, s0, s1, imm2):
    x = np.asarray(in0, np.float32)
    xl = np.asarray(in1, np.float32)
    z = x - (x == xl).astype(np.float32)
    out = np.where((x > s0) & (x <= s1), z, 0.0).astype(np.float32)
    acc = out.reshape(out.shape[0], -1).sum(axis=-1, keepdims=True).astype(np.float32)
    return out, acc


BIN_RANGE_ZSUM = _make_op(
    "BIN_RANGE_ZSUM_ANT", _body, _bin_range_zsum_ref, subdim=False, accum=AluOp.ADD
)

# ----------------------------------------------------------------------------
# bass program (one NEFF, run SPMD on 8 cores)
# ----------------------------------------------------------------------------
f16 = mybir.dt.float16
f32 = mybir.dt.float32

_NC_CACHE = {}


def _build_nc(repeats: int = 1, variant: str = "full"):
    """Raw Bass (no Tile).  repeats > 1 re-runs the identical workload
    back-to-back (for timing); every repeat recomputes the same dstat
    values, so results are unchanged.
    variant: "full" (normal), "dma" (loads only), "dve" (compute only) —
    the last two are roofline micro-benchmarks.
    """
    key = (repeats, variant)
    if key in _NC_CACHE:
        return _NC_CACHE[key]
    nc = bass.Bass()
    sm = nc.dram_tensor("sm", [T, P, FT], f16, kind="ExternalInput")
    xlab = nc.dram_tensor("xlab", [P, SPT], f16, kind="ExternalInput")
    dstat = nc.dram_tensor("dstat", [P, NG * 16], f32, kind="ExternalOutput")

    sm_v = sm.ap()

    smt = [
        nc.alloc_sbuf_tensor(f"smt{i}", [P, FT], f16).ap() for i in range(SM_BUFS)
    ]
    m1 = nc.alloc_sbuf_tensor("m1", [P, FA], f16).ap()
    m2 = nc.alloc_sbuf_tensor("m2", [P, FA // 2], f16).ap()
    m3 = nc.alloc_sbuf_tensor("m3", [P, FA // 4], f16).ap()
    m4 = nc.alloc_sbuf_tensor("m4", [P, FA // 8], f16).ap()
    conf_sb = nc.alloc_sbuf_tensor("conf_sb", [P, SPT], f16).ap()
    xlab_sb = nc.alloc_sbuf_tensor("xlab_sb", [P, SPT], f16).ap()
    dstat_sb = nc.alloc_sbuf_tensor("dstat_sb", [P, NG * 16], f32).ap()
    scrap = nc.alloc_sbuf_tensor("scrap", [P, 1], f16).ap()

    dsem = nc.alloc_semaphore()   # softmax tile DMA completions (+16 per tile)
    xsem = nc.alloc_semaphore()   # xlab in / dstat out DMA completions
    vsem = nc.alloc_semaphore()   # DVE tile consumption (+1 per sm tile)
    done = nc.alloc_semaphore()   # DVE fully done (+1 per repeat)

    do_dma = variant in ("full", "dma")
    do_dve = variant in ("full", "dve")
    gated = variant == "full"

    mx = mybir.AluOpType.max

    # ---- SP (sync) engine: aux DMAs ----
    nc.sync.dma_start(xlab_sb[:], xlab.ap()[:]).then_inc(xsem, 16)
    if gated or variant == "dve":
        nc.sync.wait_ge(done, repeats)
    elif variant == "dma":
        nc.sync.wait_ge(dsem, 32 * T * repeats)
    nc.sync.dma_start(dstat.ap()[:], dstat_sb[:]).then_inc(xsem, 16)
    nc.sync.wait_ge(xsem, 32)

    # ---- gpsimd (Pool) engine: softmax stream ----
    if do_dma:
        for r in range(repeats):
            for t in range(T):
                i = r * T + t
                if gated and i >= SM_BUFS:
                    nc.gpsimd.wait_ge(vsem, i - SM_BUFS + 1)
                elif variant == "dma" and i >= SM_BUFS:
                    # self-throttle: bound in-flight SWDGE transfers like the
                    # vsem gate does in the full variant (ring of SM_BUFS)
                    nc.gpsimd.wait_ge(dsem, 32 * (i - SM_BUFS + 1))
                buf = smt[i % SM_BUFS]
                nc.gpsimd.dma_start(buf[:, 0:FA], sm_v[t][:, 0:FA]).then_inc(
                    dsem, 16
                )
                nc.gpsimd.dma_start(buf[:, FA:FT], sm_v[t][:, FA:FT]).then_inc(
                    dsem, 16
                )

    # ---- DVE program ----
    def tt(out, a, b, op=mx):
        return nc.vector.tensor_tensor(out=out, in0=a, in1=b, op=op)

    def bins_for_group(g, tile_lo, ntiles):
        sg = ntiles * S_TILE
        lo_c = tile_lo * S_TILE
        csl = conf_sb[:, lo_c : lo_c + sg]
        xsl = xlab_sb[:, lo_c : lo_c + sg]
        inst = None
        for b in range(N_BINS):
            lo = float(BOUNDS[b])
            hi = 1.001 if b == N_BINS - 1 else float(BOUNDS[b + 1])
            inst = nc.vector._custom_dve(
                BIN_RANGE_ZSUM,
                out=scrap[:].broadcast_to([P, sg]),
                accum_out=dstat_sb[:, g * 16 + b : g * 16 + b + 1],
                in0=csl,
                in1=xsl,
                s0=lo,
                s1=hi,
            )
        return inst

    if do_dve:
        g_start = np.cumsum([0] + GROUP_TILES).tolist()
        for r in range(repeats):
            for t in range(T):
                i = r * T + t
                buf = smt[i % SM_BUFS]
                if gated:
                    nc.vector.wait_ge(dsem, 32 * (i + 1))
                lvl = tt(m1[:], buf[:, 0:FA], buf[:, FA:FT])
                lvl.then_inc(vsem, 1)
                tt(m2[:], m1[:, 0 : FA // 2], m1[:, FA // 2 : FA])
                tt(m3[:], m2[:, 0 : FA // 4], m2[:, FA // 4 : FA // 2])
                tt(m4[:], m3[:, 0 : FA // 8], m3[:, FA // 8 : FA // 4])
                tt(m1[:, 0 : FA // 16], m4[:, 0 : FA // 16],
                   m4[:, FA // 16 : FA // 8])
                cs = conf_sb[:, t * S_TILE : (t + 1) * S_TILE]
                tt(cs, m1[:, 0:S_TILE], m1[:, S_TILE : 2 * S_TILE])
                for g in range(NG):
                    if t == g_start[g + 1] - 1:
                        if r == 0 and g == 0 and gated:
                            nc.vector.wait_ge(xsem, 16)
                        inst = bins_for_group(g, g_start[g], GROUP_TILES[g])
                        if g == NG - 1:
                            inst.then_inc(done, 1)

    # Raw Bass skips this pass; without it InstCustomDveAnt/.instr stays
    # empty and walrus fails with "ISA wrong length".
    mybir.codegen_inst_isa_subclasses(nc)
    _NC_CACHE[key] = nc
    return nc


# ----------------------------------------------------------------------------
# host-side input prep (layout only: fp16 convert, permute, label gather)
# ----------------------------------------------------------------------------
def prepare_inputs(softmaxes: np.ndarray, labels: np.ndarray):
    sm16 = np.asarray(softmaxes, dtype=np.float32).astype(np.float16)
    assert sm16.shape == (N_TOTAL, C), sm16.shape
    lab = np.asarray(labels).astype(np.int64).reshape(N_TOTAL)

    xl = sm16[np.arange(N_TOTAL), lab]                       # [N] f16
    xv = np.ascontiguousarray(
        xl.reshape(CORES, T, P, S_TILE).transpose(0, 2, 1, 3)
    ).reshape(CORES, P, SPT)

    v = sm16.reshape(CORES, T, P, S_TILE, C)
    smc = np.ascontiguousarray(v.transpose(0, 1, 2, 4, 3)).reshape(CORES, T, P, FT)

    return [{"sm": smc[k], "xlab": xv[k]} for k in range(CORES)]


def finish(results) -> np.ndarray:
    d = np.zeros(N_BINS, np.float64)
    for k in range(CORES):
        st = results[k]["dstat"].astype(np.float64)
        st = st.reshape(P, NG, 16)[:, :, :N_BINS]
        d += st.sum(axis=(0, 1))
    ece = np.float32(np.abs(d).sum() / N_TOTAL)
    return np.array([ece], dtype=np.float32)


# ----------------------------------------------------------------------------
# public entry point
# ----------------------------------------------------------------------------
def kernel(softmaxes: np.ndarray, labels: np.ndarray, _want_trace=False, _repeats=1):
    nc = _build_nc(_repeats)
    in_maps = prepare_inputs(softmaxes, labels)
    res = run_bass_kernel_spmd(nc, in_maps, core_ids=list(range(CORES)))
    out = finish(res.results)
    if _want_trace:
        return out, res
    return out


# revision 15
# speedup vs baseline: 2.2712x; 1.2643x over previous
"""ECE loss kernel for Trainium2, data-parallel over 8 NeuronCores.

Strategy (v2 — fp16 streaming)
------------------------------
ECE = sum_b |sum_{i in bin b} (conf_i - acc_i)| / N, so the only binned
statistic needed per bin is d_b = sum(conf - acc).

The softmaxes are converted to fp16 on the host (rel err of the final
ECE ~1e-5, tolerance 2e-2), halving HBM traffic vs fp32.  Host also lays
each per-core stream out class-major per tile and gathers
xlab16[i] = sm16[i, label_i] (pure layout/indexing; all reduction math
stays on device).

Per core (N/8 = 524288 samples as 16 tiles of [128 part x 256 samp x 64 cl]):

1. DMA the tile (two 32-class halves, contiguous per partition).
2. DVE: 6 stock tensor_tensor MAX levels (fp16 -> 2 elem/cycle/lane)
   reduce 64 -> 1: conf16 per sample.  argmax is never materialized:
   acc_i = (conf16_i == xlab16_i) (equality of fp16 values; ~26/4.2M
   flips vs the fp32 reference argmax).
3. 15 fused BIN_RANGE_ZSUM passes per tile-group: per element,
   z = conf - (conf == xlab), accum_out[p] += z where lo < conf <= hi.
   One DVE pass per bin at 1 elem/cycle; eq/sub are folded in (free).
4. Host: sum the per-core/per-partition stats in float64, abs, sum, /N.
"""

import sys

for _p in ("/opt/trn_rl_repo",):
    if _p not in sys.path:
        sys.path.insert(0, _p)

import numpy as np

import concourse.bass as bass
import concourse.mybir as mybir
import concourse.dve_spec as ds
from concourse.dve_spec import Spec, Src0, Src1, Zero, AluOp, lower, select, eq
from concourse.dve_uop import DveOpSpec
from concourse.dve_ops import DveOp, OPS
import concourse.dve_ops as dops
from concourse.bass_utils import run_bass_kernel_spmd

# ----------------------------------------------------------------------------
# problem constants (hardcoded per the harness contract)
# ----------------------------------------------------------------------------
N_TOTAL = 4194304
C = 64
HALF = C // 2
N_BINS = 15
CORES = 8
NC_SAMP = N_TOTAL // CORES        # 524288 samples per core
P = 128                           # SBUF partitions
SPT = NC_SAMP // P                # samples per partition per core (4096)

# variable tile sizes (samples per partition); trailing tiles shrink so the
# post-stream drain (last tree + last bin group) is only a few us
TILE_S = [256] * 15 + [128, 64, 32, 32]
assert sum(TILE_S) == SPT
T = len(TILE_S)
TILE_OFF = np.cumsum([0] + TILE_S).tolist()   # sample offsets per tile
S_MAX = max(TILE_S)
FT_MAX = C * S_MAX                # elems per partition per (max) tile

# binning groups as tile ranges; the small trailing tiles get own groups
GROUP_TILES = [8, 7, 1, 1, 1, 1]
NG = len(GROUP_TILES)
SM_BUFS = 4                       # softmax tile ring depth (4 x 32KB/partition)

BOUNDS = np.linspace(0.0, 1.0, N_BINS + 1).astype(np.float32)

# ----------------------------------------------------------------------------
# custom DVE op: BIN_RANGE_ZSUM
#   out   = (C0 < Src0 <= C1) ? (Src0 - (Src0 == Src1)) : 0
#   accum_out = sum(out)     [Src0 = conf, Src1 = softmax at the true label]
# ----------------------------------------------------------------------------


def _make_op(name, spec_body, reference, subdim, accum=None):
    spec_kw = dict(body=spec_body, reference=reference)
    if accum is not None:
        spec_kw["accum"] = accum
    spec = Spec(**spec_kw)
    shas = {}
    for ver in ("v3", "v4"):
        uops = lower(spec, ver=ver)
        shas[ver] = DveOpSpec(
            name=name, opcode=0, uops=uops, rd1_en=ds._has_src1(spec)
        ).sha(ver)
    op = DveOp(name, spec, subdim=subdim, uops_sha=shas)
    if name not in dops._SUB_OPCODE_FOR_NAME:
        OPS.append(op)
        dops.CUSTOM_DVE_SPECS[name] = spec
        dops._SUB_OPCODE_FOR_NAME[name] = dops._CUSTOM_DVE_ROW_BASE + len(OPS) - 1
        assert dops._SUB_OPCODE_FOR_NAME[name] < 0x20
    else:
        op = next(o for o in OPS if o.name == name)
    return op


_inbin = ds.Bin(AluOp.LOGICAL_AND, Src0 > ds.C0, Src0 <= ds.C1)
_body = select(_inbin, Src0 - eq(Src0, Src1), Zero)


def _bin_range_zsum_ref(in0, in1, s0, s1, imm2):
    x = np.asarray(in0, np.float32)
    xl = np.asarray(in1, np.float32)
    z = x - (x == xl).astype(np.float32)
    out = np.where((x > s0) & (x <= s1), z, 0.0).astype(np.float32)
    acc = out.reshape(out.shape[0], -1).sum(axis=-1, keepdims=True).astype(np.float32)
    return out, acc


BIN_RANGE_ZSUM = _make_op(
    "BIN_RANGE_ZSUM_ANT", _body, _bin_range_zsum_ref, subdim=False, accum=AluOp.ADD
)

# ----------------------------------------------------------------------------
# bass program (one NEFF, run SPMD on 8 cores)
# ----------------------------------------------------------------------------
f16 = mybir.dt.float16
f32 = mybir.dt.float32

_NC_CACHE = {}


def _build_nc(repeats: int = 1, variant: str = "full"):
    """Raw Bass (no Tile).  repeats > 1 re-runs the identical workload
    back-to-back (for timing); every repeat recomputes the same dstat
    values, so results are unchanged.
    variant: "full" (normal), "dma" (loads only), "dve" (compute only) —
    the last two are roofline micro-benchmarks.
    """
    key = (repeats, variant)
    if key in _NC_CACHE:
        return _NC_CACHE[key]
    nc = bass.Bass()
    # per-partition row: all tiles concatenated, tile t spanning
    # C*TILE_S[t] elems at col offset C*TILE_OFF[t] (class-major per tile)
    sm = nc.dram_tensor("sm", [P, C * SPT], f16, kind="ExternalInput")
    xlab = nc.dram_tensor("xlab", [P, SPT], f16, kind="ExternalInput")
    dstat = nc.dram_tensor("dstat", [P, NG * 16], f32, kind="ExternalOutput")

    sm_v = sm.ap()

    smt = [
        nc.alloc_sbuf_tensor(f"smt{i}", [P, FT_MAX], f16).ap()
        for i in range(SM_BUFS)
    ]
    FA = HALF * S_MAX
    m1 = nc.alloc_sbuf_tensor("m1", [P, FA], f16).ap()
    m2 = nc.alloc_sbuf_tensor("m2", [P, FA // 2], f16).ap()
    m3 = nc.alloc_sbuf_tensor("m3", [P, FA // 4], f16).ap()
    m4 = nc.alloc_sbuf_tensor("m4", [P, FA // 8], f16).ap()
    conf_sb = nc.alloc_sbuf_tensor("conf_sb", [P, SPT], f16).ap()
    xlab_sb = nc.alloc_sbuf_tensor("xlab_sb", [P, SPT], f16).ap()
    dstat_sb = nc.alloc_sbuf_tensor("dstat_sb", [P, NG * 16], f32).ap()
    scrap = nc.alloc_sbuf_tensor("scrap", [P, 1], f16).ap()

    dsem = nc.alloc_semaphore()   # softmax tile DMA completions (+16 per tile)
    xsem = nc.alloc_semaphore()   # xlab in / dstat out DMA completions
    vsem = nc.alloc_semaphore()   # DVE tile consumption (+1 per sm tile)
    done = nc.alloc_semaphore()   # DVE fully done (+1 per repeat)

    do_dma = variant in ("full", "dma")
    do_dve = variant in ("full", "dve")
    gated = variant == "full"

    mx = mybir.AluOpType.max

    # ---- SP (sync) engine: aux DMAs ----
    nc.sync.dma_start(xlab_sb[:], xlab.ap()[:]).then_inc(xsem, 16)
    if gated or variant == "dve":
        nc.sync.wait_ge(done, repeats)
    elif variant == "dma":
        nc.sync.wait_ge(dsem, 16 * T * repeats)
    nc.sync.dma_start(dstat.ap()[:], dstat_sb[:]).then_inc(xsem, 16)
    nc.sync.wait_ge(xsem, 32)

    # ---- gpsimd (Pool) engine: softmax stream ----
    if do_dma:
        for r in range(repeats):
            for t in range(T):
                i = r * T + t
                if gated and i >= SM_BUFS:
                    nc.gpsimd.wait_ge(vsem, i - SM_BUFS + 1)
                elif variant == "dma" and i >= SM_BUFS:
                    # self-throttle: bound in-flight SWDGE transfers like the
                    # vsem gate does in the full variant (ring of SM_BUFS)
                    nc.gpsimd.wait_ge(dsem, 16 * (i - SM_BUFS + 1))
                buf = smt[i % SM_BUFS]
                ft = C * TILE_S[t]
                col = C * TILE_OFF[t]
                nc.gpsimd.dma_start(
                    buf[:, 0:ft], sm_v[:, col : col + ft]
                ).then_inc(dsem, 16)

    # ---- DVE program ----
    def tt(out, a, b, op=mx):
        return nc.vector.tensor_tensor(out=out, in0=a, in1=b, op=op)

    def bins_for_group(g, tile_lo, ntiles):
        sg = sum(TILE_S[tile_lo : tile_lo + ntiles])
        lo_c = TILE_OFF[tile_lo]
        csl = conf_sb[:, lo_c : lo_c + sg]
        xsl = xlab_sb[:, lo_c : lo_c + sg]
        inst = None
        for b in range(N_BINS):
            lo = float(BOUNDS[b])
            hi = 1.001 if b == N_BINS - 1 else float(BOUNDS[b + 1])
            inst = nc.vector._custom_dve(
                BIN_RANGE_ZSUM,
                out=scrap[:].broadcast_to([P, sg]),
                accum_out=dstat_sb[:, g * 16 + b : g * 16 + b + 1],
                in0=csl,
                in1=xsl,
                s0=lo,
                s1=hi,
            )
        return inst

    if do_dve:
        g_start = np.cumsum([0] + GROUP_TILES).tolist()
        for r in range(repeats):
            for t in range(T):
                i = r * T + t
                buf = smt[i % SM_BUFS]
                if gated:
                    nc.vector.wait_ge(dsem, 16 * (i + 1))
                s = TILE_S[t]
                fa = HALF * s
                lvl = tt(m1[:, 0:fa], buf[:, 0:fa], buf[:, fa : 2 * fa])
                lvl.then_inc(vsem, 1)
                tt(m2[:, 0 : fa // 2], m1[:, 0 : fa // 2], m1[:, fa // 2 : fa])
                tt(m3[:, 0 : fa // 4], m2[:, 0 : fa // 4],
                   m2[:, fa // 4 : fa // 2])
                tt(m4[:, 0 : fa // 8], m3[:, 0 : fa // 8],
                   m3[:, fa // 8 : fa // 4])
                tt(m1[:, 0 : 2 * s], m4[:, 0 : 2 * s], m4[:, 2 * s : 4 * s])
                cs = conf_sb[:, TILE_OFF[t] : TILE_OFF[t] + s]
                tt(cs, m1[:, 0:s], m1[:, s : 2 * s])
                for g in range(NG):
                    if t == g_start[g + 1] - 1:
                        if r == 0 and g == 0 and gated:
                            nc.vector.wait_ge(xsem, 16)
                        inst = bins_for_group(g, g_start[g], GROUP_TILES[g])
                        if g == NG - 1:
                            inst.then_inc(done, 1)

    # Raw Bass skips this pass; without it InstCustomDveAnt/.instr stays
    # empty and walrus fails with "ISA wrong length".
    mybir.codegen_inst_isa_subclasses(nc)
    _NC_CACHE[key] = nc
    return nc


# ----------------------------------------------------------------------------
# host-side input prep (layout only: fp16 convert, permute, label gather)
# ----------------------------------------------------------------------------
def prepare_inputs(softmaxes: np.ndarray, labels: np.ndarray):
    """Pure layout: fp16 convert, class-major permute per variable-size tile,
    label gather.  Sample n of core k lives at tile t, partition p, slot s
    with n = k*NC_SAMP + P*TILE_OFF[t] + p*TILE_S[t] + s."""
    sm16 = np.asarray(softmaxes, dtype=np.float32).astype(np.float16)
    assert sm16.shape == (N_TOTAL, C), sm16.shape
    lab = np.asarray(labels).astype(np.int64).reshape(N_TOTAL)

    xl = sm16[np.arange(N_TOTAL), lab].reshape(CORES, NC_SAMP)   # f16
    smv = sm16.reshape(CORES, NC_SAMP, C)

    smc = np.empty((CORES, P, C * SPT), np.float16)
    xv = np.empty((CORES, P, SPT), np.float16)
    for t, s in enumerate(TILE_S):
        off = TILE_OFF[t]
        blk = smv[:, P * off : P * (off + s)].reshape(CORES, P, s, C)
        smc[:, :, C * off : C * (off + s)] = (
            blk.transpose(0, 1, 3, 2).reshape(CORES, P, C * s)
        )
        xv[:, :, off : off + s] = xl[:, P * off : P * (off + s)].reshape(
            CORES, P, s
        )

    return [{"sm": smc[k], "xlab": xv[k]} for k in range(CORES)]


def finish(results) -> np.ndarray:
    d = np.zeros(N_BINS, np.float64)
    for k in range(CORES):
        st = results[k]["dstat"].astype(np.float64)
        st = st.reshape(P, NG, 16)[:, :, :N_BINS]
        d += st.sum(axis=(0, 1))
    ece = np.float32(np.abs(d).sum() / N_TOTAL)
    return np.array([ece], dtype=np.float32)


# ----------------------------------------------------------------------------
# public entry point
# ----------------------------------------------------------------------------
def kernel(softmaxes: np.ndarray, labels: np.ndarray, _want_trace=False, _repeats=1):
    nc = _build_nc(_repeats)
    in_maps = prepare_inputs(softmaxes, labels)
    res = run_bass_kernel_spmd(nc, in_maps, core_ids=list(range(CORES)))
    out = finish(res.results)
    if _want_trace:
        return out, res
    return out


# revision 20
# speedup vs baseline: 2.2841x; 1.0057x over previous
"""ECE loss kernel for Trainium2, data-parallel over 8 NeuronCores.

Strategy (v2 — fp16 streaming)
------------------------------
ECE = sum_b |sum_{i in bin b} (conf_i - acc_i)| / N, so the only binned
statistic needed per bin is d_b = sum(conf - acc).

The softmaxes are converted to fp16 on the host (rel err of the final
ECE ~1e-5, tolerance 2e-2), halving HBM traffic vs fp32.  Host also lays
each per-core stream out class-major per tile and gathers
xlab16[i] = sm16[i, label_i] (pure layout/indexing; all reduction math
stays on device).

Per core (N/8 = 524288 samples as 16 tiles of [128 part x 256 samp x 64 cl]):

1. DMA the tile (two 32-class halves, contiguous per partition).
2. DVE: 6 stock tensor_tensor MAX levels (fp16 -> 2 elem/cycle/lane)
   reduce 64 -> 1: conf16 per sample.  argmax is never materialized:
   acc_i = (conf16_i == xlab16_i) (equality of fp16 values; ~26/4.2M
   flips vs the fp32 reference argmax).
3. 15 fused BIN_RANGE_ZSUM passes per tile-group: per element,
   z = conf - (conf == xlab), accum_out[p] += z where lo < conf <= hi.
   One DVE pass per bin at 1 elem/cycle; eq/sub are folded in (free).
4. Host: sum the per-core/per-partition stats in float64, abs, sum, /N.
"""

import sys

for _p in ("/opt/trn_rl_repo",):
    if _p not in sys.path:
        sys.path.insert(0, _p)

import numpy as np

import concourse.bass as bass
import concourse.mybir as mybir
import concourse.dve_spec as ds
from concourse.dve_spec import Spec, Src0, Src1, Zero, AluOp, lower, select, eq
from concourse.dve_uop import DveOpSpec
from concourse.dve_ops import DveOp, OPS
import concourse.dve_ops as dops
from concourse.bass_utils import run_bass_kernel_spmd

# ----------------------------------------------------------------------------
# problem constants (hardcoded per the harness contract)
# ----------------------------------------------------------------------------
N_TOTAL = 4194304
C = 64
HALF = C // 2
N_BINS = 15
CORES = 8
NC_SAMP = N_TOTAL // CORES        # 524288 samples per core
P = 128                           # SBUF partitions
SPT = NC_SAMP // P                # samples per partition per core (4096)

# variable tile sizes (samples per partition); trailing tiles shrink so the
# post-stream drain (last tree + last bin group) is only a few us
TILE_S = [256] * 15 + [128, 64, 32, 32]
assert sum(TILE_S) == SPT
T = len(TILE_S)
TILE_OFF = np.cumsum([0] + TILE_S).tolist()   # sample offsets per tile
S_MAX = max(TILE_S)
FT_MAX = C * S_MAX                # elems per partition per (max) tile

# binning groups as tile ranges; the small trailing tiles get own groups
GROUP_TILES = [8, 7, 1, 1, 1, 1]
NG = len(GROUP_TILES)
SM_BUFS = 4                       # softmax tile ring depth (4 x 32KB/partition)

BOUNDS = np.linspace(0.0, 1.0, N_BINS + 1).astype(np.float32)


def configure(tile_s=None, group_tiles=None, sm_bufs=None):
    """Adjust structural knobs (benchmarking aid; defaults are the shipped
    config).  Clears the program cache."""
    global TILE_S, T, TILE_OFF, S_MAX, FT_MAX, GROUP_TILES, NG, SM_BUFS
    if tile_s is not None:
        TILE_S = list(tile_s)
        assert sum(TILE_S) == SPT
        T = len(TILE_S)
        TILE_OFF = np.cumsum([0] + TILE_S).tolist()
        S_MAX = max(TILE_S)
        FT_MAX = C * S_MAX
    if group_tiles is not None:
        GROUP_TILES = list(group_tiles)
        NG = len(GROUP_TILES)
    assert sum(GROUP_TILES) == T
    if sm_bufs is not None:
        SM_BUFS = sm_bufs
    _NC_CACHE.clear()

# ----------------------------------------------------------------------------
# custom DVE op: BIN_RANGE_ZSUM
#   out   = (C0 < Src0 <= C1) ? (Src0 - (Src0 == Src1)) : 0
#   accum_out = sum(out)     [Src0 = conf, Src1 = softmax at the true label]
# ----------------------------------------------------------------------------


def _make_op(name, spec_body, reference, subdim, accum=None):
    spec_kw = dict(body=spec_body, reference=reference)
    if accum is not None:
        spec_kw["accum"] = accum
    spec = Spec(**spec_kw)
    shas = {}
    for ver in ("v3", "v4"):
        uops = lower(spec, ver=ver)
        shas[ver] = DveOpSpec(
            name=name, opcode=0, uops=uops, rd1_en=ds._has_src1(spec)
        ).sha(ver)
    op = DveOp(name, spec, subdim=subdim, uops_sha=shas)
    if name not in dops._SUB_OPCODE_FOR_NAME:
        OPS.append(op)
        dops.CUSTOM_DVE_SPECS[name] = spec
        dops._SUB_OPCODE_FOR_NAME[name] = dops._CUSTOM_DVE_ROW_BASE + len(OPS) - 1
        assert dops._SUB_OPCODE_FOR_NAME[name] < 0x20
    else:
        op = next(o for o in OPS if o.name == name)
    return op


_inbin = ds.Bin(AluOp.LOGICAL_AND, Src0 > ds.C0, Src0 <= ds.C1)
_body = select(_inbin, Src0 - eq(Src0, Src1), Zero)


def _bin_range_zsum_ref(in0, in1, s0, s1, imm2):
    x = np.asarray(in0, np.float32)
    xl = np.asarray(in1, np.float32)
    z = x - (x == xl).astype(np.float32)
    out = np.where((x > s0) & (x <= s1), z, 0.0).astype(np.float32)
    acc = out.reshape(out.shape[0], -1).sum(axis=-1, keepdims=True).astype(np.float32)
    return out, acc


BIN_RANGE_ZSUM = _make_op(
    "BIN_RANGE_ZSUM_ANT", _body, _bin_range_zsum_ref, subdim=False, accum=AluOp.ADD
)

# ----------------------------------------------------------------------------
# bass program (one NEFF, run SPMD on 8 cores)
# ----------------------------------------------------------------------------
f16 = mybir.dt.float16
f32 = mybir.dt.float32

_NC_CACHE = {}


def _build_nc(repeats: int = 1, variant: str = "full"):
    """Raw Bass (no Tile).  repeats > 1 re-runs the identical workload
    back-to-back (for timing); every repeat recomputes the same dstat
    values, so results are unchanged.
    variant: "full" (normal), "dma" (loads only), "dve" (compute only) —
    the last two are roofline micro-benchmarks.
    """
    key = (repeats, variant)
    if key in _NC_CACHE:
        return _NC_CACHE[key]
    nc = bass.Bass()
    # per-partition row: all tiles concatenated, tile t spanning
    # C*TILE_S[t] elems at col offset C*TILE_OFF[t] (class-major per tile)
    sm = nc.dram_tensor("sm", [P, C * SPT], f16, kind="ExternalInput")
    xlab = nc.dram_tensor("xlab", [P, SPT], f16, kind="ExternalInput")
    dstat = nc.dram_tensor("dstat", [P, NG * 16], f32, kind="ExternalOutput")

    sm_v = sm.ap()

    smt = [
        nc.alloc_sbuf_tensor(f"smt{i}", [P, FT_MAX], f16).ap()
        for i in range(SM_BUFS)
    ]
    FA = HALF * S_MAX
    m1 = nc.alloc_sbuf_tensor("m1", [P, FA], f16).ap()
    m2 = nc.alloc_sbuf_tensor("m2", [P, FA // 2], f16).ap()
    m3 = nc.alloc_sbuf_tensor("m3", [P, FA // 4], f16).ap()
    m4 = nc.alloc_sbuf_tensor("m4", [P, FA // 8], f16).ap()
    conf_sb = nc.alloc_sbuf_tensor("conf_sb", [P, SPT], f16).ap()
    xlab_sb = nc.alloc_sbuf_tensor("xlab_sb", [P, SPT], f16).ap()
    dstat_sb = nc.alloc_sbuf_tensor("dstat_sb", [P, NG * 16], f32).ap()
    scrap = nc.alloc_sbuf_tensor("scrap", [P, 1], f16).ap()
    cert_sb = nc.alloc_sbuf_tensor("cert_sb", [P, 2], f16).ap()

    dsem = nc.alloc_semaphore()   # softmax tile DMA completions (+16 per tile)
    csem = nc.alloc_semaphore()   # tile certificate DMAs (+16 per tile); a
    # cert transfer queues behind its tile on the same SWDGE queue, so its 16
    # per-engine incs prove every SDMA engine finished the tile (the tile's
    # own +16 can be reached with one engine lagging while another leads)
    xsem = nc.alloc_semaphore()   # xlab in / dstat out DMA completions
    vsem = nc.alloc_semaphore()   # DVE tile consumption (+1 per sm tile)
    done = nc.alloc_semaphore()   # DVE fully done (+1 per repeat)

    do_dma = variant in ("full", "dma")
    do_dve = variant in ("full", "dve")
    gated = variant == "full"

    mx = mybir.AluOpType.max

    # ---- SP (sync) engine: aux DMAs ----
    nc.sync.dma_start(xlab_sb[:], xlab.ap()[:]).then_inc(xsem, 16)
    if gated or variant == "dve":
        nc.sync.wait_ge(done, repeats)
    elif variant == "dma":
        nc.sync.wait_ge(csem, 16 * T * repeats)
    nc.sync.dma_start(dstat.ap()[:], dstat_sb[:]).then_inc(xsem, 16)
    nc.sync.wait_ge(xsem, 32)

    # ---- gpsimd (Pool) engine: softmax stream ----
    if do_dma:
        for r in range(repeats):
            for t in range(T):
                i = r * T + t
                if gated and i >= SM_BUFS:
                    nc.gpsimd.wait_ge(vsem, i - SM_BUFS + 1)
                elif variant == "dma" and i >= SM_BUFS:
                    # self-throttle: bound in-flight SWDGE transfers like the
                    # vsem gate does in the full variant (ring of SM_BUFS)
                    nc.gpsimd.wait_ge(csem, 16 * (i - SM_BUFS + 1))
                buf = smt[i % SM_BUFS]
                ft = C * TILE_S[t]
                col = C * TILE_OFF[t]
                nc.gpsimd.dma_start(
                    buf[:, 0:ft], sm_v[:, col : col + ft]
                ).then_inc(dsem, 16)
                nc.gpsimd.dma_start(cert_sb[:], sm_v[:, 0:2]).then_inc(csem, 16)

    # ---- DVE program ----
    def tt(out, a, b, op=mx):
        return nc.vector.tensor_tensor(out=out, in0=a, in1=b, op=op)

    def bins_for_group(g, tile_lo, ntiles):
        sg = sum(TILE_S[tile_lo : tile_lo + ntiles])
        lo_c = TILE_OFF[tile_lo]
        csl = conf_sb[:, lo_c : lo_c + sg]
        xsl = xlab_sb[:, lo_c : lo_c + sg]
        inst = None
        for b in range(N_BINS):
            lo = float(BOUNDS[b])
            hi = 1.001 if b == N_BINS - 1 else float(BOUNDS[b + 1])
            inst = nc.vector._custom_dve(
                BIN_RANGE_ZSUM,
                out=scrap[:].broadcast_to([P, sg]),
                accum_out=dstat_sb[:, g * 16 + b : g * 16 + b + 1],
                in0=csl,
                in1=xsl,
                s0=lo,
                s1=hi,
            )
        return inst

    if do_dve:
        g_start = np.cumsum([0] + GROUP_TILES).tolist()
        for r in range(repeats):
            for t in range(T):
                i = r * T + t
                buf = smt[i % SM_BUFS]
                if gated:
                    nc.vector.wait_ge(csem, 16 * (i + 1))
                s = TILE_S[t]
                fa = HALF * s
                lvl = tt(m1[:, 0:fa], buf[:, 0:fa], buf[:, fa : 2 * fa])
                lvl.then_inc(vsem, 1)
                tt(m2[:, 0 : fa // 2], m1[:, 0 : fa // 2], m1[:, fa // 2 : fa])
                tt(m3[:, 0 : fa // 4], m2[:, 0 : fa // 4],
                   m2[:, fa // 4 : fa // 2])
                tt(m4[:, 0 : fa // 8], m3[:, 0 : fa // 8],
                   m3[:, fa // 8 : fa // 4])
                tt(m1[:, 0 : 2 * s], m4[:, 0 : 2 * s], m4[:, 2 * s : 4 * s])
                cs = conf_sb[:, TILE_OFF[t] : TILE_OFF[t] + s]
                tt(cs, m1[:, 0:s], m1[:, s : 2 * s])
                for g in range(NG):
                    if t == g_start[g + 1] - 1:
                        if r == 0 and g == 0 and gated:
                            nc.vector.wait_ge(xsem, 16)
                        inst = bins_for_group(g, g_start[g], GROUP_TILES[g])
                        if g == NG - 1:
                            inst.then_inc(done, 1)

    # Raw Bass skips this pass; without it InstCustomDveAnt/.instr stays
    # empty and walrus fails with "ISA wrong length".
    mybir.codegen_inst_isa_subclasses(nc)
    _NC_CACHE[key] = nc
    return nc


# ----------------------------------------------------------------------------
# host-side input prep (layout only: fp16 convert, permute, label gather)
# ----------------------------------------------------------------------------
def prepare_inputs(softmaxes: np.ndarray, labels: np.ndarray):
    """Pure layout: fp16 convert, class-major permute per variable-size tile,
    label gather.  Sample n of core k lives at tile t, partition p, slot s
    with n = k*NC_SAMP + P*TILE_OFF[t] + p*TILE_S[t] + s."""
    sm16 = np.asarray(softmaxes, dtype=np.float32).astype(np.float16)
    assert sm16.shape == (N_TOTAL, C), sm16.shape
    lab = np.asarray(labels).astype(np.int64).reshape(N_TOTAL)

    xl = sm16[np.arange(N_TOTAL), lab].reshape(CORES, NC_SAMP)   # f16
    smv = sm16.reshape(CORES, NC_SAMP, C)

    smc = np.empty((CORES, P, C * SPT), np.float16)
    xv = np.empty((CORES, P, SPT), np.float16)
    for t, s in enumerate(TILE_S):
        off = TILE_OFF[t]
        blk = smv[:, P * off : P * (off + s)].reshape(CORES, P, s, C)
        smc[:, :, C * off : C * (off + s)] = (
            blk.transpose(0, 1, 3, 2).reshape(CORES, P, C * s)
        )
        xv[:, :, off : off + s] = xl[:, P * off : P * (off + s)].reshape(
            CORES, P, s
        )

    return [{"sm": smc[k], "xlab": xv[k]} for k in range(CORES)]


def finish(results) -> np.ndarray:
    d = np.zeros(N_BINS, np.float64)
    for k in range(CORES):
        st = results[k]["dstat"].astype(np.float64)
        st = st.reshape(P, NG, 16)[:, :, :N_BINS]
        d += st.sum(axis=(0, 1))
    ece = np.float32(np.abs(d).sum() / N_TOTAL)
    return np.array([ece], dtype=np.float32)


# ----------------------------------------------------------------------------
# public entry point
# ----------------------------------------------------------------------------
def kernel(softmaxes: np.ndarray, labels: np.ndarray, _want_trace=False, _repeats=1):
    nc = _build_nc(_repeats)
    in_maps = prepare_inputs(softmaxes, labels)
    res = run_bass_kernel_spmd(nc, in_maps, core_ids=list(range(CORES)))
    out = finish(res.results)
    if _want_trace:
        return out, res
    return out


# revision 31
# speedup vs baseline: 2.9173x; 1.2772x over previous
"""ECE loss kernel for Trainium2, data-parallel over 8 NeuronCores.

Strategy (v2 — fp16 streaming)
------------------------------
ECE = sum_b |sum_{i in bin b} (conf_i - acc_i)| / N, so the only binned
statistic needed per bin is d_b = sum(conf - acc).

The softmaxes are converted to fp16 on the host (rel err of the final
ECE ~1e-5, tolerance 2e-2), halving HBM traffic vs fp32.  Host also lays
each per-core stream out class-major per tile and gathers
xlab16[i] = sm16[i, label_i] (pure layout/indexing; all reduction math
stays on device).

Per core (N/8 = 524288 samples as 16 tiles of [128 part x 256 samp x 64 cl]):

1. DMA the tile (two 32-class halves, contiguous per partition).
2. DVE: 6 stock tensor_tensor MAX levels (fp16 -> 2 elem/cycle/lane)
   reduce 64 -> 1: conf16 per sample.  argmax is never materialized:
   acc_i = (conf16_i == xlab16_i) (equality of fp16 values; ~26/4.2M
   flips vs the fp32 reference argmax).
3. 15 fused BIN_RANGE_ZSUM passes per tile-group: per element,
   z = conf - (conf == xlab), accum_out[p] += z where lo < conf <= hi.
   One DVE pass per bin at 1 elem/cycle; eq/sub are folded in (free).
4. Host: sum the per-core/per-partition stats in float64, abs, sum, /N.
"""

import sys

for _p in ("/opt/trn_rl_repo",):
    if _p not in sys.path:
        sys.path.insert(0, _p)

import numpy as np

import concourse.bass as bass
import concourse.mybir as mybir
import concourse.dve_spec as ds
from concourse.dve_spec import Spec, Src0, Src1, Zero, AluOp, lower, select, eq
from concourse.dve_uop import DveOpSpec
from concourse.dve_ops import DveOp, OPS
import concourse.dve_ops as dops
from concourse.bass_utils import run_bass_kernel_spmd

# ----------------------------------------------------------------------------
# problem constants (hardcoded per the harness contract)
# ----------------------------------------------------------------------------
N_TOTAL = 4194304
C = 64
HALF = C // 2
N_BINS = 15
CORES = 8
NC_SAMP = N_TOTAL // CORES        # 524288 samples per core
P = 128                           # SBUF partitions
SPT = NC_SAMP // P                # samples per partition per core (4096)

# tile sizes (samples per partition).  Uniform 256: small trailing tiles
# would shrink the one-shot drain tail, but their sub-1MB DMAs showed
# stale-read corruption on HW that no semaphore discipline fixed (the
# per-engine completion increments appear consolidated for tiny
# transfers), and the steady-state (repeat-slope) time is what counts.
TILE_S = [256] * 16
assert sum(TILE_S) == SPT
T = len(TILE_S)
TILE_OFF = np.cumsum([0] + TILE_S).tolist()   # sample offsets per tile
S_MAX = max(TILE_S)
FT_MAX = C * S_MAX                # elems per partition per (max) tile

# binning groups as tile ranges; small last group = short drain tail
GROUP_TILES = [8, 7, 1]
NG = len(GROUP_TILES)
SM_BUFS = 4                       # softmax tile ring depth (4 x 32KB/partition)

BOUNDS = np.linspace(0.0, 1.0, N_BINS + 1).astype(np.float32)


def configure(tile_s=None, group_tiles=None, sm_bufs=None):
    """Adjust structural knobs (benchmarking aid; defaults are the shipped
    config).  Clears the program cache."""
    global TILE_S, T, TILE_OFF, S_MAX, FT_MAX, GROUP_TILES, NG, SM_BUFS
    if tile_s is not None:
        TILE_S = list(tile_s)
        assert sum(TILE_S) == SPT
        T = len(TILE_S)
        TILE_OFF = np.cumsum([0] + TILE_S).tolist()
        S_MAX = max(TILE_S)
        FT_MAX = C * S_MAX
    if group_tiles is not None:
        GROUP_TILES = list(group_tiles)
        NG = len(GROUP_TILES)
    assert sum(GROUP_TILES) == T
    if sm_bufs is not None:
        SM_BUFS = sm_bufs
    _NC_CACHE.clear()

# ----------------------------------------------------------------------------
# custom DVE op: BIN_RANGE_ZSUM
#   out   = (C0 < Src0 <= C1) ? (Src0 - (Src0 == Src1)) : 0
#   accum_out = sum(out)     [Src0 = conf, Src1 = softmax at the true label]
# ----------------------------------------------------------------------------


def _make_op(name, spec_body, reference, subdim, accum=None):
    spec_kw = dict(body=spec_body, reference=reference)
    if accum is not None:
        spec_kw["accum"] = accum
    spec = Spec(**spec_kw)
    shas = {}
    for ver in ("v3", "v4"):
        uops = lower(spec, ver=ver)
        shas[ver] = DveOpSpec(
            name=name, opcode=0, uops=uops, rd1_en=ds._has_src1(spec)
        ).sha(ver)
    op = DveOp(name, spec, subdim=subdim, uops_sha=shas)
    if name not in dops._SUB_OPCODE_FOR_NAME:
        OPS.append(op)
        dops.CUSTOM_DVE_SPECS[name] = spec
        dops._SUB_OPCODE_FOR_NAME[name] = dops._CUSTOM_DVE_ROW_BASE + len(OPS) - 1
        assert dops._SUB_OPCODE_FOR_NAME[name] < 0x20
    else:
        op = next(o for o in OPS if o.name == name)
    return op


_inbin = ds.Bin(AluOp.LOGICAL_AND, Src0 > ds.C0, Src0 <= ds.C1)
_body = select(_inbin, Src0 - eq(Src0, Src1), Zero)


def _bin_range_zsum_ref(in0, in1, s0, s1, imm2):
    x = np.asarray(in0, np.float32)
    xl = np.asarray(in1, np.float32)
    z = x - (x == xl).astype(np.float32)
    out = np.where((x > s0) & (x <= s1), z, 0.0).astype(np.float32)
    acc = out.reshape(out.shape[0], -1).sum(axis=-1, keepdims=True).astype(np.float32)
    return out, acc


BIN_RANGE_ZSUM = _make_op(
    "BIN_RANGE_ZSUM_ANT", _body, _bin_range_zsum_ref, subdim=False, accum=AluOp.ADD
)

# ----------------------------------------------------------------------------
# bass program (one NEFF, run SPMD on 8 cores)
# ----------------------------------------------------------------------------
f16 = mybir.dt.float16
f32 = mybir.dt.float32

_NC_CACHE = {}


def _build_nc(repeats: int = 1, variant: str = "full"):
    """Raw Bass (no Tile).  repeats > 1 re-runs the identical workload
    back-to-back (for timing); every repeat recomputes the same dstat
    values, so results are unchanged.
    variant: "full" (normal), "dma" (loads only), "dve" (compute only) —
    the last two are roofline micro-benchmarks.
    """
    key = (repeats, variant)
    if key in _NC_CACHE:
        return _NC_CACHE[key]
    nc = bass.Bass()
    # per-partition row: all tiles concatenated, tile t spanning
    # C*TILE_S[t] elems at col offset C*TILE_OFF[t] (class-major per tile)
    sm = nc.dram_tensor("sm", [P, C * SPT], f16, kind="ExternalInput")
    xlab = nc.dram_tensor("xlab", [P, SPT], f16, kind="ExternalInput")
    dstat = nc.dram_tensor("dstat", [P, NG * N_BINS], f32, kind="ExternalOutput")
    debug = variant == "debug"
    if debug:
        conf_out = nc.dram_tensor("conf_out", [P, SPT], f16, kind="ExternalOutput")
        xlab_out = nc.dram_tensor("xlab_out", [P, SPT], f16, kind="ExternalOutput")
        variant = "full"

    sm_v = sm.ap()

    smt = [
        nc.alloc_sbuf_tensor(f"smt{i}", [P, FT_MAX], f16).ap()
        for i in range(SM_BUFS)
    ]
    FA = HALF * S_MAX
    m1 = nc.alloc_sbuf_tensor("m1", [P, FA], f16).ap()
    m2 = nc.alloc_sbuf_tensor("m2", [P, FA // 2], f16).ap()
    m3 = nc.alloc_sbuf_tensor("m3", [P, FA // 4], f16).ap()
    m4 = nc.alloc_sbuf_tensor("m4", [P, FA // 8], f16).ap()
    conf_sb = nc.alloc_sbuf_tensor("conf_sb", [P, SPT], f16).ap()
    xlab_sb = nc.alloc_sbuf_tensor("xlab_sb", [P, SPT], f16).ap()
    dstat_sb = nc.alloc_sbuf_tensor("dstat_sb", [P, NG * N_BINS], f32).ap()
    scrap = nc.alloc_sbuf_tensor("scrap", [P, 1], f16).ap()

    # Per-ring-slot DMA completion semaphores.  A single counting sem is
    # racy: its target count can be reached with one SDMA engine still
    # writing tile i while another engine runs a whole tile ahead (HW-
    # observed: stale-max corruption on tiles where the DVE catches up to
    # the stream).  With one sem per ring slot, reaching 16*(i//R+1) on
    # slot i%R requires tile i itself complete on all 16 engines: the
    # next same-slot tile i+R cannot contribute because its DMA is only
    # emitted after vsem >= i+1, which requires the DVE to have already
    # passed this very wait.
    dsems = [nc.alloc_semaphore(name=f"dsem{j}") for j in range(SM_BUFS)]
    xsem = nc.alloc_semaphore()   # xlab in / dstat out DMA completions
    vsem = nc.alloc_semaphore()   # DVE tile consumption (+1 per sm tile)
    done = nc.alloc_semaphore()   # DVE fully done (+1 per repeat)

    do_dma = variant in ("full", "dma")
    do_dve = variant in ("full", "dve")
    gated = variant == "full"

    mx = mybir.AluOpType.max

    # ---- SP (sync) engine: aux DMAs ----
    nc.sync.dma_start(xlab_sb[:], xlab.ap()[:]).then_inc(xsem, 16)
    if gated or variant == "dve":
        nc.sync.wait_ge(done, repeats)
    elif variant == "dma":
        n_tiles = T * repeats
        for j in range(SM_BUFS):
            cnt = (n_tiles - j + SM_BUFS - 1) // SM_BUFS
            if cnt:
                nc.sync.wait_ge(dsems[j], 16 * cnt)
    nc.sync.dma_start(dstat.ap()[:], dstat_sb[:]).then_inc(xsem, 16)
    if debug:
        nc.sync.dma_start(conf_out.ap()[:], conf_sb[:]).then_inc(xsem, 16)
        nc.sync.dma_start(xlab_out.ap()[:], xlab_sb[:]).then_inc(xsem, 16)
        nc.sync.wait_ge(xsem, 64)
    else:
        nc.sync.wait_ge(xsem, 32)

    # ---- gpsimd (Pool) engine: softmax stream ----
    if do_dma:
        for r in range(repeats):
            for t in range(T):
                i = r * T + t
                if gated and i >= SM_BUFS:
                    nc.gpsimd.wait_ge(vsem, i - SM_BUFS + 1)
                elif variant == "dma" and i >= SM_BUFS:
                    # self-throttle: bound in-flight SWDGE transfers like the
                    # vsem gate does in the full variant (ring of SM_BUFS)
                    nc.gpsimd.wait_ge(dsems[i % SM_BUFS], 16 * (i // SM_BUFS))
                buf = smt[i % SM_BUFS]
                ft = C * TILE_S[t]
                col = C * TILE_OFF[t]
                nc.gpsimd.dma_start(
                    buf[:, 0:ft], sm_v[:, col : col + ft]
                ).then_inc(dsems[i % SM_BUFS], 16)

    # ---- DVE program ----
    def tt(out, a, b, op=mx):
        return nc.vector.tensor_tensor(out=out, in0=a, in1=b, op=op)

    def bins_for_group(g, tile_lo, ntiles):
        sg = sum(TILE_S[tile_lo : tile_lo + ntiles])
        lo_c = TILE_OFF[tile_lo]
        csl = conf_sb[:, lo_c : lo_c + sg]
        xsl = xlab_sb[:, lo_c : lo_c + sg]
        inst = None
        for b in range(N_BINS):
            lo = float(BOUNDS[b])
            hi = 1.001 if b == N_BINS - 1 else float(BOUNDS[b + 1])
            inst = nc.vector._custom_dve(
                BIN_RANGE_ZSUM,
                out=scrap[:].broadcast_to([P, sg]),
                accum_out=dstat_sb[:, g * N_BINS + b : g * N_BINS + b + 1],
                in0=csl,
                in1=xsl,
                s0=lo,
                s1=hi,
            )
        return inst

    if do_dve:
        g_start = np.cumsum([0] + GROUP_TILES).tolist()
        for r in range(repeats):
            for t in range(T):
                i = r * T + t
                buf = smt[i % SM_BUFS]
                if gated:
                    nc.vector.wait_ge(dsems[i % SM_BUFS], 16 * (i // SM_BUFS + 1))
                s = TILE_S[t]
                fa = HALF * s
                lvl = tt(m1[:, 0:fa], buf[:, 0:fa], buf[:, fa : 2 * fa])
                lvl.then_inc(vsem, 1)
                tt(m2[:, 0 : fa // 2], m1[:, 0 : fa // 2], m1[:, fa // 2 : fa])
                tt(m3[:, 0 : fa // 4], m2[:, 0 : fa // 4],
                   m2[:, fa // 4 : fa // 2])
                tt(m4[:, 0 : fa // 8], m3[:, 0 : fa // 8],
                   m3[:, fa // 8 : fa // 4])
                tt(m1[:, 0 : 2 * s], m4[:, 0 : 2 * s], m4[:, 2 * s : 4 * s])
                cs = conf_sb[:, TILE_OFF[t] : TILE_OFF[t] + s]
                tt(cs, m1[:, 0:s], m1[:, s : 2 * s])
                for g in range(NG):
                    if t == g_start[g + 1] - 1:
                        if r == 0 and g == 0 and gated:
                            nc.vector.wait_ge(xsem, 16)
                        inst = bins_for_group(g, g_start[g], GROUP_TILES[g])
                        if g == NG - 1:
                            inst.then_inc(done, 1)

    # Raw Bass skips this pass; without it InstCustomDveAnt/.instr stays
    # empty and walrus fails with "ISA wrong length".
    mybir.codegen_inst_isa_subclasses(nc)
    _NC_CACHE[key] = nc
    return nc


# ----------------------------------------------------------------------------
# host-side input prep (layout only: fp16 convert, permute, label gather)
# ----------------------------------------------------------------------------
def prepare_inputs(softmaxes: np.ndarray, labels: np.ndarray):
    """Pure layout: fp16 convert, class-major permute per variable-size tile,
    label gather.  Sample n of core k lives at tile t, partition p, slot s
    with n = k*NC_SAMP + P*TILE_OFF[t] + p*TILE_S[t] + s."""
    sm16 = np.asarray(softmaxes, dtype=np.float32).astype(np.float16)
    assert sm16.shape == (N_TOTAL, C), sm16.shape
    lab = np.asarray(labels).astype(np.int64).reshape(N_TOTAL)

    xl = sm16[np.arange(N_TOTAL), lab].reshape(CORES, NC_SAMP)   # f16
    smv = sm16.reshape(CORES, NC_SAMP, C)

    smc = np.empty((CORES, P, C * SPT), np.float16)
    xv = np.empty((CORES, P, SPT), np.float16)
    for t, s in enumerate(TILE_S):
        off = TILE_OFF[t]
        blk = smv[:, P * off : P * (off + s)].reshape(CORES, P, s, C)
        smc[:, :, C * off : C * (off + s)] = (
            blk.transpose(0, 1, 3, 2).reshape(CORES, P, C * s)
        )
        xv[:, :, off : off + s] = xl[:, P * off : P * (off + s)].reshape(
            CORES, P, s
        )

    return [{"sm": smc[k], "xlab": xv[k]} for k in range(CORES)]


def finish(results) -> np.ndarray:
    d = np.zeros(N_BINS, np.float64)
    for k in range(CORES):
        st = results[k]["dstat"].astype(np.float64)
        st = st.reshape(P, NG, N_BINS)
        d += st.sum(axis=(0, 1))
    ece = np.float32(np.abs(d).sum() / N_TOTAL)
    return np.array([ece], dtype=np.float32)


# ----------------------------------------------------------------------------
# public entry point
# ----------------------------------------------------------------------------
def kernel(softmaxes: np.ndarray, labels: np.ndarray, _want_trace=False, _repeats=1):
    nc = _build_nc(_repeats)
    in_maps = prepare_inputs(softmaxes, labels)
    res = run_bass_kernel_spmd(nc, in_maps, core_ids=list(range(CORES)))
    out = finish(res.results)
    if _want_trace:
        return out, res
    return out
